# revision 21
# baseline (speedup 1.0000x reference)
"""Trainium2 Bass kernel for nn_Prop_padding (vq_codebook).

Data-parallel over 8 NeuronCores: 128 samples/core. Within a core the batch
is processed in segments; conv layers run as block-diagonal sample-group
packed matmuls (K = g*Cin stacked on partitions) with tap accumulation in
PSUM. All matmuls in bf16 (fp32 PSUM accumulate).
"""

import math
import os
import sys

for _p in ("/opt/trn_rl_repo",):
    if _p not in sys.path:
        sys.path.insert(0, _p)

import numpy as np
import ml_dtypes

import concourse.bass as bass
import concourse.bacc as bacc
import concourse.mybir as mybir
import concourse.tile as tile
from concourse.bass_utils import run_bass_kernel_spmd

F32 = mybir.dt.float32
BF16 = mybir.dt.bfloat16
AF = mybir.ActivationFunctionType

N_CORES = 8
B_CORE = 128
FILT = 40
D_S = 15
F2 = 50
NUM_S = 32
NUM_N = 10

# mid interleaved 20-ch block offsets (j: stream = j%2, e-block = j//2)
MID_OFFS = [0, 20, 40, 60, 80, 100]
# 15-ch block offsets after mc (cs blocks contiguous at 0, cn at 64)
M1_OFFS_CS = [0, 15, 30]
M1_OFFS_CN = [64, 79, 94]
DEC_OFFS = [0, 32, 64, 96]


# ---------------------------------------------------------------- host prep

def _bd(W, blocks, K):
    """Block-diag lhsT from conv weight W (O, I, k): returns (K, k, 128)."""
    W = np.asarray(W, np.float32)
    O, I, k = W.shape
    out = np.zeros((K, k, 128), np.float32)
    for io, oo in blocks:
        for t in range(k):
            out[io:io + I, t, oo:oo + O] = W[:, :, t].T
    return out.astype(ml_dtypes.bfloat16)


GAP50 = [c if c < 25 else 32 + (c - 25) for c in range(50)]  # 50ch w/ gap


def _bd_map(W, blocks, K):
    """blocks: list of (in_rows[I], out_cols[O]) index arrays."""
    W = np.asarray(W, np.float32)
    O, I, k = W.shape
    out = np.zeros((K, k, 128), np.float32)
    for irows, ocols in blocks:
        for t in range(k):
            for i in range(I):
                out[irows[i], t, ocols] = W[:, i, t]
    return out.astype(ml_dtypes.bfloat16)


def _bias_map(b, maps):
    out = np.zeros((128, 1), np.float32)
    b = np.asarray(b, np.float32)
    for rows in maps:
        out[rows, 0] = b
    return out


def _bias(b, offs, O):
    out = np.zeros((128, 1), np.float32)
    b = np.asarray(b, np.float32)
    for oo in offs:
        out[oo:oo + O, 0] = b
    return out


def prep_weights(params):
    p = params
    H = {}

    def blocks(offs_in, offs_out):
        return list(zip(offs_in, offs_out))

    e_offs = [0, 40, 80]
    H["w_e0c1"] = _bd(p["e0"]["W1"], blocks([0, 1, 2], e_offs), 3)
    H["b_e0c1"] = _bias(p["e0"]["b1"], e_offs, FILT)
    H["w_e0c2"] = _bd(p["e0"]["W2"], blocks(e_offs, e_offs), 120)
    H["b_e0c2"] = _bias(p["e0"]["b2"], e_offs, FILT)
    H["w_sr"] = _bd(p["sr_W"], blocks(e_offs, e_offs), 120)
    H["b_sr"] = _bias(p["sr_b"], e_offs, FILT)
    for nm in ("e1", "e2"):
        H[f"w_{nm}c1"] = _bd(p[nm]["W1"], blocks(e_offs, e_offs), 120)
        H[f"b_{nm}c1"] = _bias(p[nm]["b1"], e_offs, FILT)
        H[f"w_{nm}c2"] = _bd(p[nm]["W2"], blocks(e_offs, e_offs), 120)
        H[f"b_{nm}c2"] = _bias(p[nm]["b2"], e_offs, FILT)

    H["w_m0c1"] = _bd(p["m0"]["W1"], blocks(MID_OFFS, MID_OFFS), 120)
    H["b_m0c1"] = _bias(p["m0"]["b1"], MID_OFFS, 20)
    H["w_m0c2"] = _bd(p["m0"]["W2"], blocks(MID_OFFS, MID_OFFS), 120)
    H["b_m0c2"] = _bias(p["m0"]["b2"], MID_OFFS, 20)
    # mc: interleaved in, de-interleaved out (cs j -> 15j ; cn j -> 64+15j)
    mc_out = [M1_OFFS_CS[0], M1_OFFS_CN[0], M1_OFFS_CS[1], M1_OFFS_CN[1],
              M1_OFFS_CS[2], M1_OFFS_CN[2]]
    H["w_mc"] = _bd(p["mc_W"], blocks(MID_OFFS, mc_out), 120)
    H["b_mc"] = _bias(p["mc_b"], mc_out, D_S)
    m1_offs = M1_OFFS_CS + M1_OFFS_CN
    H["w_m1c1"] = _bd(p["m1"]["W1"], blocks(m1_offs, m1_offs), 109)
    H["b_m1c1"] = _bias(p["m1"]["b1"], m1_offs, D_S)
    H["w_m1c2"] = _bd(p["m1"]["W2"], blocks(m1_offs, m1_offs), 109)
    H["b_m1c2"] = _bias(p["m1"]["b2"], m1_offs, D_S)

    # code-assign weights per stream
    for st, mkey, M in (("s", "means_s", NUM_S), ("n", "means_n", NUM_N)):
        means = np.asarray(p[mkey], np.float32)          # (15, M)
        wd = np.zeros((45, 3 * M), np.float32)
        bd_ = np.zeros((3 * M, 1), np.float32)
        ws = np.zeros((3 * M, 3), np.float32)
        wb = np.zeros((3, 3 * M), np.float32)
        wq = np.zeros((3 * M, 96), np.float32)
        msq = -np.sum(means * means, axis=0)
        for j in range(3):
            wd[15 * j:15 * j + 15, M * j:M * j + M] = 2.0 * means
            bd_[M * j:M * j + M, 0] = msq
            ws[M * j:M * j + M, j] = 1.0
            wb[j, M * j:M * j + M] = 1.0
            wq[M * j:M * j + M, 32 * j:32 * j + 15] = means.T
        H[f"w_dist_{st}"] = wd.astype(ml_dtypes.bfloat16)
        H[f"b_dist_{st}"] = bd_
        H[f"w_sum_{st}"] = ws.astype(ml_dtypes.bfloat16)
        H[f"w_bc_{st}"] = wb.astype(ml_dtypes.bfloat16)
        H[f"w_q_{st}"] = wq.astype(ml_dtypes.bfloat16)

    # ai: per stream, 2 sample blocks (15 in at {0,32} -> 50 out at {0,64});
    # cs uses ch 15:30, cn 0:15
    aiW = np.asarray(p["ai_W"], np.float32)              # (50, 30, 3)
    H["w_ai_s"] = _bd(aiW[:, 15:30, :], blocks([0, 32], [0, 64]), 47)
    H["w_ai_n"] = _bd(aiW[:, 0:15, :], blocks([0, 32], [0, 64]), 47)
    H["b_ai"] = _bias(p["ai_b"], [0, 64], F2)

    # ab: stream blocks at {0, 64}, 50 contiguous channels
    H["w_abc1"] = _bd(p["ab"]["W1"], blocks([0, 64], [0, 64]), 114)
    H["b_abc1"] = _bias(p["ab"]["b1"], [0, 64], F2)
    H["w_abc2"] = _bd(p["ab"]["W2"], blocks([0, 64], [0, 64]), 114)
    H["b_abc2"] = _bias(p["ab"]["b2"], [0, 64], F2)

    # ao: sample blocks at {0, 64}, channels in gap layout (cs 0-24, cn 32-56)
    g50 = np.array(GAP50)
    ao_blocks = [(g50, g50), (g50 + 64, g50 + 64)]
    ao_bmap = [g50, g50 + 64]
    for nm in ("ao0", "ao1"):
        H[f"w_{nm}c1"] = _bd_map(p[nm]["W1"], ao_blocks, 121)
        H[f"b_{nm}c1"] = _bias_map(p[nm]["b1"], ao_bmap)
        H[f"w_{nm}c2"] = _bd_map(p[nm]["W2"], ao_blocks, 121)
        H[f"b_{nm}c2"] = _bias_map(p[nm]["b2"], ao_bmap)

    for nm in ("d0", "d1"):
        H[f"w_{nm}c1"] = _bd(p[nm]["W1"], blocks(DEC_OFFS, DEC_OFFS), 121)
        H[f"b_{nm}c1"] = _bias(p[nm]["b1"], DEC_OFFS, 25)
        H[f"w_{nm}c2"] = _bd(p[nm]["W2"], blocks(DEC_OFFS, DEC_OFFS), 121)
        H[f"b_{nm}c2"] = _bias(p[nm]["b2"], DEC_OFFS, 25)

    H["fc_Wt"] = np.asarray(p["fc_W"], np.float32).reshape(100, 128, 512) \
        .astype(ml_dtypes.bfloat16)
    H["fc_b"] = np.asarray(p["fc_b"], np.float32).reshape(1, 512) \
        .astype(ml_dtypes.bfloat16)
    H["ones_1"] = np.ones((1, 128), ml_dtypes.bfloat16)
    H["ident"] = np.eye(128, dtype=np.float32).astype(ml_dtypes.bfloat16)
    return H


# ---------------------------------------------------------------- builder

class KB:
    """Kernel builder state."""

    def __init__(self, segs, bseg):
        self.segs = segs
        self.bseg = bseg
        self.esl = math.ceil(bseg / 3)    # slots per e/mid block
        self.hb = bseg // 2               # ao/dec block size
        self.bcore = segs * bseg


def build_nc(segs=4, bseg=32):
    kb = KB(segs, bseg)
    nc = bacc.Bacc()
    ESL, HB, BSEG = kb.esl, kb.hb, kb.bseg

    # dram io
    import contextlib  # noqa
    x_d = nc.dram_tensor("x", [kb.bcore, 512], BF16, kind="ExternalInput")
    shapes = {
        "w_e0c1": [3, 3, 128], "w_e0c2": [120, 3, 128], "w_sr": [120, 5, 128],
        "w_e1c1": [120, 3, 128], "w_e1c2": [120, 3, 128],
        "w_e2c1": [120, 3, 128], "w_e2c2": [120, 3, 128],
        "w_m0c1": [120, 3, 128], "w_m0c2": [120, 3, 128],
        "w_mc": [120, 5, 128],
        "w_m1c1": [109, 3, 128], "w_m1c2": [109, 3, 128],
        "w_dist_s": [45, 96], "w_sum_s": [96, 3], "w_bc_s": [3, 96],
        "w_q_s": [96, 96],
        "w_dist_n": [45, 30], "w_sum_n": [30, 3], "w_bc_n": [3, 30],
        "w_q_n": [30, 96],
        "w_ai_s": [47, 3, 128], "w_ai_n": [47, 3, 128],
        "w_abc1": [114, 3, 128], "w_abc2": [114, 3, 128],
        "w_ao0c1": [121, 3, 128], "w_ao0c2": [121, 3, 128],
        "w_ao1c1": [121, 3, 128], "w_ao1c2": [121, 3, 128],
        "w_d0c1": [121, 3, 128], "w_d0c2": [121, 3, 128],
        "w_d1c1": [121, 3, 128], "w_d1c2": [121, 3, 128],
        "fc_Wt": [100, 128, 512], "fc_b": [1, 512], "ones_1": [1, 128],
        "ident": [128, 128],
    }
    bias_shapes = {
        "b_e0c1": [128, 1], "b_e0c2": [128, 1], "b_sr": [128, 1],
        "b_e1c1": [128, 1], "b_e1c2": [128, 1], "b_e2c1": [128, 1],
        "b_e2c2": [128, 1], "b_m0c1": [128, 1], "b_m0c2": [128, 1],
        "b_mc": [128, 1], "b_m1c1": [128, 1], "b_m1c2": [128, 1],
        "b_dist_s": [96, 1], "b_dist_n": [30, 1], "b_ai": [128, 1],
        "b_abc1": [128, 1], "b_abc2": [128, 1],
        "b_ao0c1": [128, 1], "b_ao0c2": [128, 1],
        "b_ao1c1": [128, 1], "b_ao1c2": [128, 1],
        "b_d0c1": [128, 1], "b_d0c2": [128, 1],
        "b_d1c1": [128, 1], "b_d1c2": [128, 1],
    }
    dram = {}
    for nm, sh in shapes.items():
        dram[nm] = nc.dram_tensor(nm, sh, BF16, kind="ExternalInput")
    for nm, sh in bias_shapes.items():
        dram[nm] = nc.dram_tensor(nm, sh, F32, kind="ExternalInput")

    dec_s_d = nc.dram_tensor("dec_s", [kb.bcore, 512], F32, kind="ExternalOutput")
    dec_n_d = nc.dram_tensor("dec_n", [kb.bcore, 512], F32, kind="ExternalOutput")
    prob_s_d = nc.dram_tensor("prob_s", [NUM_S, kb.bcore, 256], F32,
                              kind="ExternalOutput")
    prob_n_d = nc.dram_tensor("prob_n", [NUM_N, kb.bcore, 256], F32,
                              kind="ExternalOutput")

    with tile.TileContext(nc) as tc:
        import contextlib
        ctx = contextlib.ExitStack()
        with ctx:
            wpool = ctx.enter_context(tc.tile_pool(name="wpool", bufs=1))
            acts = ctx.enter_context(tc.tile_pool(name="acts", bufs=6))
            code_sb = ctx.enter_context(tc.tile_pool(name="code_sb", bufs=4))
            tmp_p = ctx.enter_context(tc.tile_pool(name="tmp", bufs=2))
            att_p = ctx.enter_context(tc.tile_pool(name="att", bufs=1))
            wfc_p = ctx.enter_context(tc.tile_pool(name="wfc", bufs=3))
            osb_p = ctx.enter_context(tc.tile_pool(name="osb", bufs=2))
            conv_ps = ctx.enter_context(
                tc.tile_pool(name="conv_ps", bufs=3, space="PSUM"))
            zq_ps = ctx.enter_context(
                tc.tile_pool(name="zq_ps", bufs=2, space="PSUM"))
            misc_ps = ctx.enter_context(
                tc.tile_pool(name="misc_ps", bufs=3, space="PSUM"))

            # ---- load weights to sbuf
            W = {}
            for nm in list(shapes) + list(bias_shapes):
                sh = shapes.get(nm) or bias_shapes[nm]
                dt = BF16 if nm in shapes else F32
                if nm == "fc_Wt":
                    continue  # streamed later
                if nm == "w_dist_n":
                    t = wpool.tile([128, 30], BF16, tag=nm)
                    nc.sync.dma_start(out=t[64:109, :], in_=dram[nm][:, :])
                    W[nm] = t
                    continue
                t = wpool.tile(sh, dt, tag=nm)
                sl = tuple(slice(0, s) for s in sh)
                nc.sync.dma_start(out=t[sl], in_=dram[nm][sl])
                W[nm] = t

            actT = {st: att_p.tile([128, 100 * 128], BF16, tag=f"actT_{st}",
                                         name=f"actT_{st}")
                    for st in ("s", "n")}

            # ------------------------------------------------ conv helper
            def conv(src, wname, K, taps, Lout, nslots, chunk_slots, stride,
                     writes, skip=None, skip_rows=128):
                """writes: list of (r0, r1, dstfn(s0, ns)->AP, bias_ap, relu)"""
                nchunks = math.ceil(nslots / chunk_slots)
                w_t = W[wname]
                for c in range(nchunks):
                    s0 = c * chunk_slots
                    ns = min(chunk_slots, nslots - s0)
                    ps = conv_ps.tile([128, chunk_slots, Lout], F32, tag="cps")
                    for t in range(taps):
                        if stride == 1:
                            rhs = src[0:K, s0:s0 + ns, t:t + Lout]
                        else:
                            rhs = src[0:K, s0:s0 + ns, t:t + Lout * stride:stride]
                        nc.tensor.matmul(ps[:, 0:ns, :], w_t[0:K, t, :], rhs,
                                         start=(t == 0), stop=(t == taps - 1))
                    src_c = ps
                    if skip is not None:
                        sk_t, sk_pad = skip
                        tmp = tmp_p.tile([128, chunk_slots, Lout], F32, tag="tmp")
                        nc.vector.tensor_add(
                            out=tmp[0:skip_rows, 0:ns, :],
                            in0=ps[0:skip_rows, 0:ns, :],
                            in1=sk_t[0:skip_rows, s0:s0 + ns,
                                     sk_pad:sk_pad + Lout])
                        src_c = tmp
                    for (r0, r1, dstfn, bias_ap, relu) in writes:
                        nc.scalar.activation(
                            dstfn(s0, ns), src_c[r0:r1, 0:ns, :],
                            AF.Relu if relu else AF.Identity,
                            bias=bias_ap, scale=1.0)

            def wfull(dst, pad, Lout, r0=0, r1=128, d0=None, bname=None,
                      relu=True, slot_off=0):
                d0 = r0 if d0 is None else d0
                nr = r1 - r0
                bias = W[bname][r0:r1, 0:1]
                return (r0, r1,
                        lambda s0, ns, _d=dst, _p=pad, _L=Lout, _d0=d0, _nr=nr,
                               _so=slot_off:
                        _d[_d0:_d0 + _nr, _so + s0:_so + s0 + ns, _p:_p + _L],
                        bias, relu)

            # pad-zero memset helper
            def pads(t, rows, nslots, slotw, pad):
                if pad == 1:
                    nc.vector.memset(
                        t[0:rows, 0:nslots, 0:slotw:slotw - 1], 0.0)
                else:
                    nc.vector.memset(t[0:rows, 0:nslots, 0:pad], 0.0)
                    nc.vector.memset(
                        t[0:rows, 0:nslots, slotw - pad:slotw], 0.0)

            # ------------------------------------------------ segments
            for seg in range(segs):
                sb = seg * BSEG  # sample base

                def new_act(name, parts, slots, slotw, pad):
                    t = acts.tile([parts, slots, slotw], BF16, tag="act",
                                  name=name)
                    if pad:
                        pads(t, parts, slots, slotw, pad)
                    return t

                x_t = new_act("x_t", 3, ESL, 514, 1)
                # zero garbage slots first (NaN x 0 = NaN would poison the
                # shared columns of the other blocks in block-diag matmuls)
                if 3 * ESL > BSEG:
                    gs = BSEG - 2 * ESL
                    nc.vector.memset(x_t[0:3, gs:ESL, :], 0.0)

                # load x: per e-block, clipped
                for j in range(3):
                    cnt = min(ESL, BSEG - j * ESL)
                    nc.sync.dma_start(
                        out=x_t[j:j + 1, 0:cnt, 1:513],
                        in_=x_d[sb + j * ESL: sb + j * ESL + cnt, :]
                        .unsqueeze(0))

                # encoder
                e0a = new_act("e0a", 128, ESL, 514, 1)
                conv(x_t, "w_e0c1", 3, 3, 512, ESL, 1, 1,
                     [wfull(e0a, 1, 512, bname="b_e0c1")])
                e0b = new_act("e0b", 128, ESL, 516, 2)
                conv(e0a, "w_e0c2", 120, 3, 512, ESL, 1, 1,
                     [wfull(e0b, 2, 512, bname="b_e0c2")])
                sr_o = new_act("sr_o", 128, ESL, 258, 1)
                conv(e0b, "w_sr", 120, 5, 256, ESL, 2, 2,
                     [wfull(sr_o, 1, 256, bname="b_sr")])
                e1a = new_act("e1a", 128, ESL, 258, 1)
                conv(sr_o, "w_e1c1", 120, 3, 256, ESL, 2, 1,
                     [wfull(e1a, 1, 256, bname="b_e1c1")])
                e1b = new_act("e1b", 128, ESL, 258, 1)
                conv(e1a, "w_e1c2", 120, 3, 256, ESL, 2, 1,
                     [wfull(e1b, 1, 256, bname="b_e1c2")], skip=(sr_o, 1))
                e2a = new_act("e2a", 128, ESL, 258, 1)
                conv(e1b, "w_e2c1", 120, 3, 256, ESL, 2, 1,
                     [wfull(e2a, 1, 256, bname="b_e2c1")])
                mid_in = new_act("mid_in", 128, ESL, 258, 1)
                conv(e2a, "w_e2c2", 120, 3, 256, ESL, 2, 1,
                     [wfull(mid_in, 1, 256, bname="b_e2c2")], skip=(e1b, 1))

                # mid
                m0a = new_act("m0a", 128, ESL, 258, 1)
                conv(mid_in, "w_m0c1", 120, 3, 256, ESL, 2, 1,
                     [wfull(m0a, 1, 256, bname="b_m0c1")])
                m0b = new_act("m0b", 128, ESL, 260, 2)
                conv(m0a, "w_m0c2", 120, 3, 256, ESL, 2, 1,
                     [wfull(m0b, 2, 256, bname="b_m0c2")], skip=(mid_in, 1))
                mc_o = new_act("mc_o", 128, ESL, 258, 1)
                conv(m0b, "w_mc", 120, 5, 256, ESL, 2, 1,
                     [wfull(mc_o, 1, 256, bname="b_mc")])
                m1a = new_act("m1a", 128, ESL, 258, 1)
                conv(mc_o, "w_m1c1", 109, 3, 256, ESL, 2, 1,
                     [wfull(m1a, 1, 256, bname="b_m1c1")])
                m1b = new_act("m1b", 128, ESL, 258, 1)
                conv(m1a, "w_m1c2", 109, 3, 256, ESL, 2, 1,
                     [wfull(m1b, 1, 256, bname="b_m1c2")], skip=(mc_o, 1))

                ai_cs = new_act("ai_cs", 64, HB, 258, 1)
                ai_cn = new_act("ai_cn", 64, HB, 258, 1)

                # ---- code assign per stream
                for st, M, base, ai_t, prob_d in (
                        ("s", NUM_S, 0, ai_cs, prob_s_d),
                        ("n", NUM_N, 64, ai_cn, prob_n_d)):
                    M3 = 3 * M
                    nch = math.ceil(ESL / 2)
                    for c in range(nch):
                        s0 = 2 * c
                        ns = min(2, ESL - s0)
                        ps_z = zq_ps.tile([M3, 2, 256], F32, tag="zq")
                        lhs = (W["w_dist_s"][0:45, :] if st == "s"
                               else W["w_dist_n"][64:109, :])
                        nc.tensor.matmul(
                            ps_z[:, 0:ns, :], lhs,
                            m1b[base:base + 45, s0:s0 + ns, 1:257],
                            start=True, stop=True)
                        exp_t = code_sb.tile([M3, 2, 256], BF16, tag="code")
                        nc.scalar.activation(
                            exp_t[:, 0:ns, :], ps_z[:, 0:ns, :], AF.Exp,
                            bias=W[f"b_dist_{st}"][0:M3, 0:1], scale=1.0)
                        ps_sum = misc_ps.tile([3, 2, 256], F32, tag="mps")
                        nc.tensor.matmul(ps_sum[:, 0:ns, :],
                                         W[f"w_sum_{st}"][0:M3, :],
                                         exp_t[:, 0:ns, :],
                                         start=True, stop=True)
                        rec_t = code_sb.tile([3, 2, 256], BF16, tag="code")
                        with nc.allow_low_precision(
                                reason="softmax recip broadcast in bf16"):
                            nc.vector.reciprocal(rec_t[:, 0:ns, :],
                                                 ps_sum[:, 0:ns, :])
                        ps_bc = misc_ps.tile([M3, 2, 256], F32, tag="mps")
                        nc.tensor.matmul(ps_bc[:, 0:ns, :],
                                         W[f"w_bc_{st}"][0:3, :],
                                         rec_t[:, 0:ns, :],
                                         start=True, stop=True)
                        prob_f = code_sb.tile([M3, 2, 256], F32, tag="code")
                        nc.vector.tensor_mul(out=prob_f[:, 0:ns, :],
                                             in0=exp_t[:, 0:ns, :],
                                             in1=ps_bc[:, 0:ns, :])
                        prob_b = code_sb.tile([M3, 2, 256], BF16, tag="code")
                        nc.vector.tensor_mul(out=prob_b[:, 0:ns, :],
                                             in0=exp_t[:, 0:ns, :],
                                             in1=ps_bc[:, 0:ns, :])
                        ps_q = zq_ps.tile([96, 2, 256], F32, tag="zq")
                        nc.tensor.matmul(ps_q[:, 0:ns, :],
                                         W[f"w_q_{st}"][0:M3, :],
                                         prob_b[:, 0:ns, :],
                                         start=True, stop=True)
                        # prob out DMA + q scatter, per block, clipped
                        for j3 in range(3):
                            runs = []
                            for s in range(s0, s0 + ns):
                                m = j3 * ESL + s
                                if m >= BSEG:
                                    continue
                                runs.append((s, m))
                            if not runs:
                                continue
                            rs, m0_ = runs[0]
                            cnt = len(runs)
                            nc.sync.dma_start(
                                out=prob_d[:, sb + m0_: sb + m0_ + cnt, :],
                                in_=prob_f[M * j3:M * j3 + M,
                                           rs - s0:rs - s0 + cnt, :])
                            # q -> ai tile (block m//HB, slot m%HB); split runs
                            i = 0
                            while i < cnt:
                                m = m0_ + i
                                blk = m // HB
                                n2 = min(cnt - i, (blk + 1) * HB - m)
                                nc.vector.tensor_copy(
                                    out=ai_t[32 * blk:32 * blk + 32,
                                             (m % HB):(m % HB) + n2, 1:257],
                                    in_=ps_q[32 * j3:32 * j3 + 32,
                                             rs - s0 + i:rs - s0 + i + n2, :])
                                i += n2

                # ---- ai convs (per stream) -> ab_in
                ab_in = new_act("ab_in", 128, BSEG, 258, 1)
                for st, ai_t, r_ab in (("s", ai_cs, 0), ("n", ai_cn, 64)):
                    conv(ai_t, f"w_ai_{st}", 47, 3, 256, HB, 2, 1,
                         [(0, 64, lambda s0, ns, _r=r_ab:
                           ab_in[_r:_r + 64, s0:s0 + ns, 1:257],
                           W["b_ai"][0:64, 0:1], False),
                          (64, 128, lambda s0, ns, _r=r_ab:
                           ab_in[_r:_r + 64, HB + s0:HB + s0 + ns, 1:257],
                           W["b_ai"][64:128, 0:1], False)])

                ab_a = new_act("ab_a", 128, BSEG, 258, 1)
                conv(ab_in, "w_abc1", 114, 3, 256, BSEG, 2, 1,
                     [wfull(ab_a, 1, 256, bname="b_abc1")])
                ab_o = new_act("ab_o", 128, BSEG, 258, 0)
                conv(ab_a, "w_abc2", 114, 3, 256, BSEG, 2, 1,
                     [wfull(ab_o, 1, 256, bname="b_abc2")],
                     skip=(ab_in, 1))

                # ---- sub_pixel: ab_o -> ao_in (per-slot DMAs)
                ao_in = acts.tile([128, HB, 514], BF16, tag="act",
                                  name="ao_in")
                nc.vector.memset(ao_in[:, :, :], 0.0)
                for strm in (0, 1):
                    for j in (0, 1):
                        for k in (0, 1):
                            for s in range(HB):
                                nc.sync.dma_start(
                                    out=ao_in[64 * k + 32 * strm:
                                              64 * k + 32 * strm + 25,
                                              s, 1 + j:1 + j + 512:2],
                                    in_=ab_o[64 * strm + j:
                                             64 * strm + 50:2,
                                             HB * k + s, 1:257])

                ao0a = new_act("ao0a", 128, HB, 514, 1)
                conv(ao_in, "w_ao0c1", 121, 3, 512, HB, 1, 1,
                     [wfull(ao0a, 1, 512, bname="b_ao0c1")])
                ao0b = new_act("ao0b", 128, HB, 514, 1)
                conv(ao0a, "w_ao0c2", 121, 3, 512, HB, 1, 1,
                     [wfull(ao0b, 1, 512, bname="b_ao0c2")],
                     skip=(ao_in, 1))
                ao1a = new_act("ao1a", 128, HB, 514, 1)
                conv(ao0b, "w_ao1c1", 121, 3, 512, HB, 1, 1,
                     [wfull(ao1a, 1, 512, bname="b_ao1c1")])
                # ao1c2: 4-way scatter into dec_in
                dec_in = acts.tile([128, HB, 514], BF16, tag="act",
                                   name="dec_in")
                nc.vector.memset(dec_in[:, :, :], 0.0)
                conv(ao1a, "w_ao1c2", 121, 3, 512, HB, 1, 1,
                     [(0, 25, lambda s0, ns:
                       dec_in[0:25, s0:s0 + ns, 1:513],
                       W["b_ao1c2"][0:25, 0:1], True),
                      (32, 57, lambda s0, ns:
                       dec_in[64:89, s0:s0 + ns, 1:513],
                       W["b_ao1c2"][32:57, 0:1], True),
                      (64, 89, lambda s0, ns:
                       dec_in[32:57, s0:s0 + ns, 1:513],
                       W["b_ao1c2"][64:89, 0:1], True),
                      (96, 121, lambda s0, ns:
                       dec_in[96:121, s0:s0 + ns, 1:513],
                       W["b_ao1c2"][96:121, 0:1], True)],
                     skip=(ao0b, 1))

                d0a = new_act("d0a", 128, HB, 514, 1)
                conv(dec_in, "w_d0c1", 121, 3, 512, HB, 1, 1,
                     [wfull(d0a, 1, 512, bname="b_d0c1")])
                d0b = new_act("d0b", 128, HB, 514, 1)
                conv(d0a, "w_d0c2", 121, 3, 512, HB, 1, 1,
                     [wfull(d0b, 1, 512, bname="b_d0c2")], skip=(dec_in, 1))
                d1a = new_act("d1a", 128, HB, 514, 1)
                conv(d0b, "w_d1c1", 121, 3, 512, HB, 1, 1,
                     [wfull(d1a, 1, 512, bname="b_d1c1")])
                d1b = new_act("d1b", 128, HB, 514, 0)
                conv(d1a, "w_d1c2", 121, 3, 512, HB, 1, 1,
                     [wfull(d1b, 1, 512, bname="b_d1c2")], skip=(d0b, 1))

                # ---- transposes into actT
                for s in range(HB):
                    for lb in range(4):
                        ps_t = misc_ps.tile([128, 128], BF16, tag="mps")
                        nc.tensor.transpose(
                            ps_t[:, :], d1b[0:128, s, 1 + 128 * lb:
                                            1 + 128 * (lb + 1)],
                            W["ident"][:, :])
                        src4 = ps_t.rearrange("p (b c) -> p c b", c=32)
                        for sti, st in ((0, "s"), (1, "n")):
                            base_r = lb * 128 + sb + s
                            dst = actT[st].rearrange(
                                "p (c r) -> p c r", r=512)[
                                :, 0:25, base_r:base_r + HB + 1:HB]
                            nc.vector.tensor_copy(
                                out=dst,
                                in_=src4[:, 0:25, 2 * sti:2 * sti + 2])

            # ---------------- fc
            ps_fc = {st: conv_ps.tile([128, 512], F32, tag="cps",
                                          name=f"psfc_{st}")
                     for st in ("s", "n")}
            for kc in range(100):
                wt = wfc_p.tile([128, 512], BF16, tag="fcw")
                nc.sync.dma_start(out=wt[:, :], in_=dram["fc_Wt"][kc, :, :])
                for st in ("s", "n"):
                    nc.tensor.matmul(ps_fc[st][:, :],
                                     actT[st][:, kc * 128:(kc + 1) * 128],
                                     wt[:, :], start=(kc == 0), stop=False,
                                     skip_group_check=True)
            fcb_t = wpool.tile([1, 512], BF16, tag="fcb")
            nc.sync.dma_start(out=fcb_t[:, :], in_=dram["fc_b"][:, :])
            for st, od in (("s", dec_s_d), ("n", dec_n_d)):
                nc.tensor.matmul(ps_fc[st][:, :], W["ones_1"][0:1, :],
                                 fcb_t[:, :], start=False, stop=True,
                                 skip_group_check=True)
                o_t = osb_p.tile([128, 512], F32, tag="osb")
                nc.scalar.activation(o_t[:, :], ps_fc[st][:, :], AF.Tanh)
                nc.sync.dma_start(out=od[0:kb.bcore, :],
                                  in_=o_t[0:kb.bcore, :])

    nc.finalize()
    return nc, kb


# ---------------------------------------------------------------- runner

_CACHE = {}


def _get_nc():
    if "nc" not in _CACHE:
        _CACHE["nc"] = build_nc(4, 32)
    return _CACHE["nc"]


def kernel(x, params):
    x = np.asarray(x, np.float32)
    nc, kb = _get_nc()
    H = prep_weights(params)
    in_maps = []
    for c in range(N_CORES):
        m = dict(H)
        m["x"] = x[c * B_CORE:(c + 1) * B_CORE].astype(ml_dtypes.bfloat16)
        in_maps.append(m)
    trace = os.environ.get("KERNEL_TRACE") == "1"
    res = run_bass_kernel_spmd(nc, in_maps, core_ids=list(range(N_CORES)),
                               trace=trace)
    _CACHE["last_res"] = res
    dec_s = np.concatenate([res.results[c]["dec_s"] for c in range(N_CORES)])
    dec_n = np.concatenate([res.results[c]["dec_n"] for c in range(N_CORES)])
    prob_s = np.concatenate(
        [np.transpose(res.results[c]["prob_s"], (1, 2, 0))
         for c in range(N_CORES)])
    prob_n = np.concatenate(
        [np.transpose(res.results[c]["prob_n"], (1, 2, 0))
         for c in range(N_CORES)])
    return dec_s, dec_n, prob_s, prob_n


# revision 30
# speedup vs baseline: 9.5566x; 9.5566x over previous
"""Trainium2 Bass kernel for nn_Prop_padding (vq_codebook).

Data-parallel over 8 NeuronCores: 128 samples/core. Within a core the batch
is processed in segments; conv layers run as block-diagonal sample-group
packed matmuls (K = g*Cin stacked on partitions) with tap accumulation in
PSUM. All matmuls in bf16 (fp32 PSUM accumulate).
"""

import math
import os
import sys

for _p in ("/opt/trn_rl_repo",):
    if _p not in sys.path:
        sys.path.insert(0, _p)

import numpy as np
import ml_dtypes

import concourse.bass as bass
import concourse.bacc as bacc
import concourse.mybir as mybir
import concourse.tile as tile
from concourse.bass_utils import run_bass_kernel_spmd

F32 = mybir.dt.float32
BF16 = mybir.dt.bfloat16
AF = mybir.ActivationFunctionType

N_CORES = 8
B_CORE = 128
FILT = 40
D_S = 15
F2 = 50
NUM_S = 32
NUM_N = 10

# mid interleaved 20-ch block offsets (j: stream = j%2, e-block = j//2)
MID_OFFS = [0, 20, 40, 60, 80, 100]
# 15-ch block offsets after mc (cs blocks contiguous at 0, cn at 64)
M1_OFFS_CS = [0, 15, 30]
M1_OFFS_CN = [64, 79, 94]
DEC_OFFS = [0, 32, 64, 96]


# ---------------------------------------------------------------- host prep

def _bd(W, blocks, K):
    """Block-diag lhsT from conv weight W (O, I, k): returns (K, k, 128)."""
    W = np.asarray(W, np.float32)
    O, I, k = W.shape
    out = np.zeros((K, k, 128), np.float32)
    for io, oo in blocks:
        for t in range(k):
            out[io:io + I, t, oo:oo + O] = W[:, :, t].T
    return out.astype(ml_dtypes.bfloat16)


GAP50 = [c if c < 25 else 32 + (c - 25) for c in range(50)]  # 50ch w/ gap


def _bd_map(W, blocks, K):
    """blocks: list of (in_rows[I], out_cols[O]) index arrays."""
    W = np.asarray(W, np.float32)
    O, I, k = W.shape
    out = np.zeros((K, k, 128), np.float32)
    for irows, ocols in blocks:
        for t in range(k):
            for i in range(I):
                out[irows[i], t, ocols] = W[:, i, t]
    return out.astype(ml_dtypes.bfloat16)


def _bias_map(b, maps):
    out = np.zeros((128, 1), np.float32)
    b = np.asarray(b, np.float32)
    for rows in maps:
        out[rows, 0] = b
    return out


def _bias(b, offs, O):
    out = np.zeros((128, 1), np.float32)
    b = np.asarray(b, np.float32)
    for oo in offs:
        out[oo:oo + O, 0] = b
    return out


def prep_weights(params):
    p = params
    H = {}

    def blocks(offs_in, offs_out):
        return list(zip(offs_in, offs_out))

    e_offs = [0, 40, 80]
    H["w_e0c1"] = _bd(p["e0"]["W1"], blocks([0, 1, 2], e_offs), 3)
    H["b_e0c1"] = _bias(p["e0"]["b1"], e_offs, FILT)
    H["w_e0c2"] = _bd(p["e0"]["W2"], blocks(e_offs, e_offs), 120)
    H["b_e0c2"] = _bias(p["e0"]["b2"], e_offs, FILT)
    H["w_sr"] = _bd(p["sr_W"], blocks(e_offs, e_offs), 120)
    H["b_sr"] = _bias(p["sr_b"], e_offs, FILT)
    for nm in ("e1", "e2"):
        H[f"w_{nm}c1"] = _bd(p[nm]["W1"], blocks(e_offs, e_offs), 120)
        H[f"b_{nm}c1"] = _bias(p[nm]["b1"], e_offs, FILT)
        H[f"w_{nm}c2"] = _bd(p[nm]["W2"], blocks(e_offs, e_offs), 120)
        H[f"b_{nm}c2"] = _bias(p[nm]["b2"], e_offs, FILT)

    H["w_m0c1"] = _bd(p["m0"]["W1"], blocks(MID_OFFS, MID_OFFS), 120)
    H["b_m0c1"] = _bias(p["m0"]["b1"], MID_OFFS, 20)
    H["w_m0c2"] = _bd(p["m0"]["W2"], blocks(MID_OFFS, MID_OFFS), 120)
    H["b_m0c2"] = _bias(p["m0"]["b2"], MID_OFFS, 20)
    # mc: interleaved in, de-interleaved out (cs j -> 15j ; cn j -> 64+15j)
    mc_out = [M1_OFFS_CS[0], M1_OFFS_CN[0], M1_OFFS_CS[1], M1_OFFS_CN[1],
              M1_OFFS_CS[2], M1_OFFS_CN[2]]
    H["w_mc"] = _bd(p["mc_W"], blocks(MID_OFFS, mc_out), 120)
    H["b_mc"] = _bias(p["mc_b"], mc_out, D_S)
    m1_offs = M1_OFFS_CS + M1_OFFS_CN
    H["w_m1c1"] = _bd(p["m1"]["W1"], blocks(m1_offs, m1_offs), 109)
    H["b_m1c1"] = _bias(p["m1"]["b1"], m1_offs, D_S)
    H["w_m1c2"] = _bd(p["m1"]["W2"], blocks(m1_offs, m1_offs), 109)
    H["b_m1c2"] = _bias(p["m1"]["b2"], m1_offs, D_S)

    # code-assign weights per stream
    for st, mkey, M in (("s", "means_s", NUM_S), ("n", "means_n", NUM_N)):
        means = np.asarray(p[mkey], np.float32)          # (15, M)
        wd = np.zeros((45, 3 * M), np.float32)
        bd_ = np.zeros((3 * M, 1), np.float32)
        ws = np.zeros((3 * M, 3), np.float32)
        wb = np.zeros((3, 3 * M), np.float32)
        wq = np.zeros((3 * M, 96), np.float32)
        msq = -np.sum(means * means, axis=0)
        for j in range(3):
            wd[15 * j:15 * j + 15, M * j:M * j + M] = 2.0 * means
            bd_[M * j:M * j + M, 0] = msq
            ws[M * j:M * j + M, j] = 1.0
            wb[j, M * j:M * j + M] = 1.0
            wq[M * j:M * j + M, 32 * j:32 * j + 15] = means.T
        H[f"w_dist_{st}"] = wd.astype(ml_dtypes.bfloat16)
        H[f"b_dist_{st}"] = bd_
        H[f"w_sum_{st}"] = ws.astype(ml_dtypes.bfloat16)
        H[f"w_bc_{st}"] = wb.astype(ml_dtypes.bfloat16)
        H[f"w_q_{st}"] = wq.astype(ml_dtypes.bfloat16)

    # ai: per stream, 2 sample blocks (15 in at {0,32} -> 50 out at {0,64});
    # cs uses ch 15:30, cn 0:15
    aiW = np.asarray(p["ai_W"], np.float32)              # (50, 30, 3)
    H["w_ai_s"] = _bd(aiW[:, 15:30, :], blocks([0, 32], [0, 64]), 47)
    H["w_ai_n"] = _bd(aiW[:, 0:15, :], blocks([0, 32], [0, 64]), 47)
    H["b_ai"] = _bias(p["ai_b"], [0, 64], F2)

    # ab: stream blocks at {0, 64}, 50 contiguous channels
    H["w_abc1"] = _bd(p["ab"]["W1"], blocks([0, 64], [0, 64]), 114)
    H["b_abc1"] = _bias(p["ab"]["b1"], [0, 64], F2)
    H["w_abc2"] = _bd(p["ab"]["W2"], blocks([0, 64], [0, 64]), 114)
    H["b_abc2"] = _bias(p["ab"]["b2"], [0, 64], F2)

    # ao: sample blocks at {0, 64}, channels in gap layout (cs 0-24, cn 32-56)
    g50 = np.array(GAP50)
    ao_blocks = [(g50, g50), (g50 + 64, g50 + 64)]
    ao_bmap = [g50, g50 + 64]
    for nm in ("ao0", "ao1"):
        H[f"w_{nm}c1"] = _bd_map(p[nm]["W1"], ao_blocks, 121)
        H[f"b_{nm}c1"] = _bias_map(p[nm]["b1"], ao_bmap)
        H[f"w_{nm}c2"] = _bd_map(p[nm]["W2"], ao_blocks, 121)
        H[f"b_{nm}c2"] = _bias_map(p[nm]["b2"], ao_bmap)

    for nm in ("d0", "d1"):
        H[f"w_{nm}c1"] = _bd(p[nm]["W1"], blocks(DEC_OFFS, DEC_OFFS), 121)
        H[f"b_{nm}c1"] = _bias(p[nm]["b1"], DEC_OFFS, 25)
        H[f"w_{nm}c2"] = _bd(p[nm]["W2"], blocks(DEC_OFFS, DEC_OFFS), 121)
        H[f"b_{nm}c2"] = _bias(p[nm]["b2"], DEC_OFFS, 25)

    H["fc_Wt"] = np.asarray(p["fc_W"], np.float32).reshape(100, 128, 512) \
        .astype(ml_dtypes.bfloat16)
    H["fc_b"] = np.asarray(p["fc_b"], np.float32).reshape(1, 512) \
        .astype(ml_dtypes.bfloat16)
    H["ones_1"] = np.ones((1, 128), ml_dtypes.bfloat16)
    H["ident"] = np.eye(128, dtype=np.float32).astype(ml_dtypes.bfloat16)
    return H


# ---------------------------------------------------------------- builder

class KB:
    """Kernel builder state."""

    def __init__(self, segs, bseg):
        self.segs = segs
        self.bseg = bseg
        self.esl = math.ceil(bseg / 3)    # slots per e/mid block
        self.hb = bseg // 2               # ao/dec block size
        self.bcore = segs * bseg


def build_nc(segs=4, bseg=32, repeat=1):
    kb = KB(segs, bseg)
    nc = bacc.Bacc()
    ESL, HB, BSEG = kb.esl, kb.hb, kb.bseg

    # dram io
    import contextlib  # noqa
    x_d = nc.dram_tensor("x", [kb.bcore, 512], BF16, kind="ExternalInput")
    shapes = {
        "w_e0c1": [3, 3, 128], "w_e0c2": [120, 3, 128], "w_sr": [120, 5, 128],
        "w_e1c1": [120, 3, 128], "w_e1c2": [120, 3, 128],
        "w_e2c1": [120, 3, 128], "w_e2c2": [120, 3, 128],
        "w_m0c1": [120, 3, 128], "w_m0c2": [120, 3, 128],
        "w_mc": [120, 5, 128],
        "w_m1c1": [109, 3, 128], "w_m1c2": [109, 3, 128],
        "w_dist_s": [45, 96], "w_sum_s": [96, 3], "w_bc_s": [3, 96],
        "w_q_s": [96, 96],
        "w_dist_n": [45, 30], "w_sum_n": [30, 3], "w_bc_n": [3, 30],
        "w_q_n": [30, 96],
        "w_ai_s": [47, 3, 128], "w_ai_n": [47, 3, 128],
        "w_abc1": [114, 3, 128], "w_abc2": [114, 3, 128],
        "w_ao0c1": [121, 3, 128], "w_ao0c2": [121, 3, 128],
        "w_ao1c1": [121, 3, 128], "w_ao1c2": [121, 3, 128],
        "w_d0c1": [121, 3, 128], "w_d0c2": [121, 3, 128],
        "w_d1c1": [121, 3, 128], "w_d1c2": [121, 3, 128],
        "fc_Wt": [100, 128, 512], "fc_b": [1, 512], "ones_1": [1, 128],
        "ident": [128, 128],
    }
    bias_shapes = {
        "b_e0c1": [128, 1], "b_e0c2": [128, 1], "b_sr": [128, 1],
        "b_e1c1": [128, 1], "b_e1c2": [128, 1], "b_e2c1": [128, 1],
        "b_e2c2": [128, 1], "b_m0c1": [128, 1], "b_m0c2": [128, 1],
        "b_mc": [128, 1], "b_m1c1": [128, 1], "b_m1c2": [128, 1],
        "b_dist_s": [96, 1], "b_dist_n": [30, 1], "b_ai": [128, 1],
        "b_abc1": [128, 1], "b_abc2": [128, 1],
        "b_ao0c1": [128, 1], "b_ao0c2": [128, 1],
        "b_ao1c1": [128, 1], "b_ao1c2": [128, 1],
        "b_d0c1": [128, 1], "b_d0c2": [128, 1],
        "b_d1c1": [128, 1], "b_d1c2": [128, 1],
    }
    dram = {}
    for nm, sh in shapes.items():
        dram[nm] = nc.dram_tensor(nm, sh, BF16, kind="ExternalInput")
    for nm, sh in bias_shapes.items():
        dram[nm] = nc.dram_tensor(nm, sh, F32, kind="ExternalInput")

    dec_s_d = nc.dram_tensor("dec_s", [kb.bcore, 512], F32, kind="ExternalOutput")
    dec_n_d = nc.dram_tensor("dec_n", [kb.bcore, 512], F32, kind="ExternalOutput")
    prob_s_d = nc.dram_tensor("prob_s", [NUM_S, kb.bcore, 256], F32,
                              kind="ExternalOutput")
    prob_n_d = nc.dram_tensor("prob_n", [NUM_N, kb.bcore, 256], F32,
                              kind="ExternalOutput")

    with tile.TileContext(nc) as tc:
        import contextlib
        ctx = contextlib.ExitStack()
        with ctx:
            wpool = ctx.enter_context(tc.tile_pool(name="wpool", bufs=1))
            acts = ctx.enter_context(tc.tile_pool(name="acts", bufs=6))
            code_sb = ctx.enter_context(tc.tile_pool(name="code_sb", bufs=6))
            tmp_p = ctx.enter_context(tc.tile_pool(name="tmp", bufs=3))
            att_p = ctx.enter_context(tc.tile_pool(name="att", bufs=1))
            wfc_p = ctx.enter_context(tc.tile_pool(name="wfc", bufs=3))
            osb_p = ctx.enter_context(tc.tile_pool(name="osb", bufs=2))
            conv_ps = ctx.enter_context(
                tc.tile_pool(name="conv_ps", bufs=4, space="PSUM"))
            zq_ps = ctx.enter_context(
                tc.tile_pool(name="zq_ps", bufs=2, space="PSUM"))
            misc_ps = ctx.enter_context(
                tc.tile_pool(name="misc_ps", bufs=2, space="PSUM"))

            # ---- load weights to sbuf
            W = {}
            for nm in list(shapes) + list(bias_shapes):
                sh = shapes.get(nm) or bias_shapes[nm]
                dt = BF16 if nm in shapes else F32
                if nm == "fc_Wt":
                    continue  # streamed later
                if nm == "w_dist_n":
                    t = wpool.tile([128, 30], BF16, tag=nm)
                    nc.sync.dma_start(out=t[64:109, :], in_=dram[nm][:, :])
                    W[nm] = t
                    continue
                t = wpool.tile(sh, dt, tag=nm)
                sl = tuple(slice(0, s) for s in sh)
                nc.sync.dma_start(out=t[sl], in_=dram[nm][sl])
                W[nm] = t

            actT = {st: att_p.tile([128, 100 * 128], BF16, tag=f"actT_{st}",
                                         name=f"actT_{st}")
                    for st in ("s", "n")}

            # ------------------------------------------------ conv helper
            def conv(src, wname, K, taps, Lout, nslots, chunk_slots, stride,
                     writes, skip=None, skip_rows=128):
                """writes: list of (r0, r1, dstfn(s0, ns)->AP, bias_ap, relu)"""
                nchunks = math.ceil(nslots / chunk_slots)
                conv._alt += 1
                w_t = W[wname]
                for c in range(nchunks):
                    s0 = c * chunk_slots
                    ns = min(chunk_slots, nslots - s0)
                    ps = conv_ps.tile([128, chunk_slots, Lout], F32, tag="cps")
                    for t in range(taps):
                        if stride == 1:
                            rhs = src[0:K, s0:s0 + ns, t:t + Lout]
                        else:
                            rhs = src[0:K, s0:s0 + ns, t:t + Lout * stride:stride]
                        nc.tensor.matmul(ps[:, 0:ns, :], w_t[0:K, t, :], rhs,
                                         start=(t == 0), stop=(t == taps - 1))
                    src_c = ps
                    if skip is not None:
                        sk_t, sk_pad = skip
                        tmp = tmp_p.tile([128, chunk_slots, Lout], F32, tag="tmp")
                        nc.vector.tensor_add(
                            out=tmp[0:skip_rows, 0:ns, :],
                            in0=ps[0:skip_rows, 0:ns, :],
                            in1=sk_t[0:skip_rows, s0:s0 + ns,
                                     sk_pad:sk_pad + Lout])
                        src_c = tmp
                    use_dve = False
                    for (r0, r1, dstfn, bias_ap, relu) in writes:
                        if use_dve:
                            nc.vector.tensor_scalar(
                                out=dstfn(s0, ns),
                                in0=src_c[r0:r1, 0:ns, :],
                                scalar1=bias_ap,
                                scalar2=0.0 if relu else None,
                                op0=mybir.AluOpType.add,
                                op1=(mybir.AluOpType.max if relu
                                     else mybir.AluOpType.bypass))
                        else:
                            nc.scalar.activation(
                                dstfn(s0, ns), src_c[r0:r1, 0:ns, :],
                                AF.Relu if relu else AF.Identity,
                                bias=bias_ap, scale=1.0)

            conv._alt = 0

            def wfull(dst, pad, Lout, r0=0, r1=128, d0=None, bname=None,
                      relu=True, slot_off=0):
                d0 = r0 if d0 is None else d0
                nr = r1 - r0
                bias = W[bname][r0:r1, 0:1]
                return (r0, r1,
                        lambda s0, ns, _d=dst, _p=pad, _L=Lout, _d0=d0, _nr=nr,
                               _so=slot_off:
                        _d[_d0:_d0 + _nr, _so + s0:_so + s0 + ns, _p:_p + _L],
                        bias, relu)

            # pad-zero memset helper
            def pads(t, rows, nslots, slotw, pad):
                if pad == 1:
                    nc.vector.memset(
                        t[0:rows, 0:nslots, 0:slotw:slotw - 1], 0.0)
                else:
                    nc.vector.memset(t[0:rows, 0:nslots, 0:pad], 0.0)
                    nc.vector.memset(
                        t[0:rows, 0:nslots, slotw - pad:slotw], 0.0)

            # ------------------------------------------------ segments
            for _rep in range(repeat):
              for seg in range(segs):
                sb = seg * BSEG  # sample base

                def new_act(name, parts, slots, slotw, pad):
                    t = acts.tile([parts, slots, slotw], BF16, tag="act",
                                  name=name)
                    if pad:
                        pads(t, parts, slots, slotw, pad)
                    return t

                x_t = new_act("x_t", 3, ESL, 514, 1)
                # zero garbage slots first (NaN x 0 = NaN would poison the
                # shared columns of the other blocks in block-diag matmuls)
                if 3 * ESL > BSEG:
                    gs = BSEG - 2 * ESL
                    nc.vector.memset(x_t[0:3, gs:ESL, :], 0.0)

                # load x: per e-block, clipped
                for j in range(3):
                    cnt = min(ESL, BSEG - j * ESL)
                    nc.sync.dma_start(
                        out=x_t[j:j + 1, 0:cnt, 1:513],
                        in_=x_d[sb + j * ESL: sb + j * ESL + cnt, :]
                        .unsqueeze(0))

                # encoder
                e0a = new_act("e0a", 128, ESL, 514, 1)
                conv(x_t, "w_e0c1", 3, 3, 512, ESL, 1, 1,
                     [wfull(e0a, 1, 512, bname="b_e0c1")])
                e0b = new_act("e0b", 128, ESL, 516, 2)
                conv(e0a, "w_e0c2", 120, 3, 512, ESL, 1, 1,
                     [wfull(e0b, 2, 512, bname="b_e0c2")])
                sr_o = new_act("sr_o", 128, ESL, 258, 1)
                conv(e0b, "w_sr", 120, 5, 256, ESL, 2, 2,
                     [wfull(sr_o, 1, 256, bname="b_sr")])
                e1a = new_act("e1a", 128, ESL, 258, 1)
                conv(sr_o, "w_e1c1", 120, 3, 256, ESL, 2, 1,
                     [wfull(e1a, 1, 256, bname="b_e1c1")])
                e1b = new_act("e1b", 128, ESL, 258, 1)
                conv(e1a, "w_e1c2", 120, 3, 256, ESL, 2, 1,
                     [wfull(e1b, 1, 256, bname="b_e1c2")], skip=(sr_o, 1))
                e2a = new_act("e2a", 128, ESL, 258, 1)
                conv(e1b, "w_e2c1", 120, 3, 256, ESL, 2, 1,
                     [wfull(e2a, 1, 256, bname="b_e2c1")])
                mid_in = new_act("mid_in", 128, ESL, 258, 1)
                conv(e2a, "w_e2c2", 120, 3, 256, ESL, 2, 1,
                     [wfull(mid_in, 1, 256, bname="b_e2c2")], skip=(e1b, 1))

                # mid
                m0a = new_act("m0a", 128, ESL, 258, 1)
                conv(mid_in, "w_m0c1", 120, 3, 256, ESL, 2, 1,
                     [wfull(m0a, 1, 256, bname="b_m0c1")])
                m0b = new_act("m0b", 128, ESL, 260, 2)
                conv(m0a, "w_m0c2", 120, 3, 256, ESL, 2, 1,
                     [wfull(m0b, 2, 256, bname="b_m0c2")], skip=(mid_in, 1))
                mc_o = new_act("mc_o", 128, ESL, 258, 1)
                conv(m0b, "w_mc", 120, 5, 256, ESL, 2, 1,
                     [wfull(mc_o, 1, 256, bname="b_mc")])
                m1a = new_act("m1a", 128, ESL, 258, 1)
                conv(mc_o, "w_m1c1", 109, 3, 256, ESL, 2, 1,
                     [wfull(m1a, 1, 256, bname="b_m1c1")])
                m1b = new_act("m1b", 128, ESL, 258, 1)
                conv(m1a, "w_m1c2", 109, 3, 256, ESL, 2, 1,
                     [wfull(m1b, 1, 256, bname="b_m1c2")], skip=(mc_o, 1))

                ai_cs = new_act("ai_cs", 64, HB, 258, 1)
                ai_cn = new_act("ai_cn", 64, HB, 258, 1)

                # ---- code assign per stream
                for st, M, base, ai_t, prob_d in (
                        ("s", NUM_S, 0, ai_cs, prob_s_d),
                        ("n", NUM_N, 64, ai_cn, prob_n_d)):
                    M3 = 3 * M
                    nch = math.ceil(ESL / 2)
                    for c in range(nch):
                        s0 = 2 * c
                        ns = min(2, ESL - s0)
                        ps_z = zq_ps.tile([M3, 2, 256], F32, tag="zq")
                        lhs = (W["w_dist_s"][0:45, :] if st == "s"
                               else W["w_dist_n"][64:109, :])
                        nc.tensor.matmul(
                            ps_z[:, 0:ns, :], lhs,
                            m1b[base:base + 45, s0:s0 + ns, 1:257],
                            start=True, stop=True)
                        exp_t = code_sb.tile([M3, 2, 256], BF16, tag="code")
                        nc.scalar.activation(
                            exp_t[:, 0:ns, :], ps_z[:, 0:ns, :], AF.Exp,
                            bias=W[f"b_dist_{st}"][0:M3, 0:1], scale=1.0)
                        ps_sum = misc_ps.tile([3, 2, 256], F32, tag="mps")
                        nc.tensor.matmul(ps_sum[:, 0:ns, :],
                                         W[f"w_sum_{st}"][0:M3, :],
                                         exp_t[:, 0:ns, :],
                                         start=True, stop=True)
                        rec_t = code_sb.tile([3, 2, 256], BF16, tag="code")
                        with nc.allow_low_precision(
                                reason="softmax recip broadcast in bf16"):
                            nc.vector.reciprocal(rec_t[:, 0:ns, :],
                                                 ps_sum[:, 0:ns, :])
                        ps_bc = misc_ps.tile([M3, 2, 256], F32, tag="mps")
                        nc.tensor.matmul(ps_bc[:, 0:ns, :],
                                         W[f"w_bc_{st}"][0:3, :],
                                         rec_t[:, 0:ns, :],
                                         start=True, stop=True)
                        prob_f = code_sb.tile([M3, 2, 256], F32, tag="code")
                        nc.vector.tensor_mul(out=prob_f[:, 0:ns, :],
                                             in0=exp_t[:, 0:ns, :],
                                             in1=ps_bc[:, 0:ns, :])
                        prob_b = code_sb.tile([M3, 2, 256], BF16, tag="code")
                        nc.vector.tensor_mul(out=prob_b[:, 0:ns, :],
                                             in0=exp_t[:, 0:ns, :],
                                             in1=ps_bc[:, 0:ns, :])
                        ps_q = zq_ps.tile([96, 2, 256], F32, tag="zq")
                        nc.tensor.matmul(ps_q[:, 0:ns, :],
                                         W[f"w_q_{st}"][0:M3, :],
                                         prob_b[:, 0:ns, :],
                                         start=True, stop=True)
                        # prob out DMA + q scatter, per block, clipped
                        for j3 in range(3):
                            runs = []
                            for s in range(s0, s0 + ns):
                                m = j3 * ESL + s
                                if m >= BSEG:
                                    continue
                                runs.append((s, m))
                            if not runs:
                                continue
                            rs, m0_ = runs[0]
                            cnt = len(runs)
                            nc.sync.dma_start(
                                out=prob_d[:, sb + m0_: sb + m0_ + cnt, :],
                                in_=prob_f[M * j3:M * j3 + M,
                                           rs - s0:rs - s0 + cnt, :])
                            # q -> ai tile (block m//HB, slot m%HB); split runs
                            i = 0
                            while i < cnt:
                                m = m0_ + i
                                blk = m // HB
                                n2 = min(cnt - i, (blk + 1) * HB - m)
                                nc.vector.tensor_copy(
                                    out=ai_t[32 * blk:32 * blk + 32,
                                             (m % HB):(m % HB) + n2, 1:257],
                                    in_=ps_q[32 * j3:32 * j3 + 32,
                                             rs - s0 + i:rs - s0 + i + n2, :])
                                i += n2

                # ---- ai convs (per stream) -> ab_in
                ab_in = new_act("ab_in", 128, BSEG, 258, 1)
                for st, ai_t, r_ab in (("s", ai_cs, 0), ("n", ai_cn, 64)):
                    conv(ai_t, f"w_ai_{st}", 47, 3, 256, HB, 2, 1,
                         [(0, 64, lambda s0, ns, _r=r_ab:
                           ab_in[_r:_r + 64, s0:s0 + ns, 1:257],
                           W["b_ai"][0:64, 0:1], False),
                          (64, 128, lambda s0, ns, _r=r_ab:
                           ab_in[_r:_r + 64, HB + s0:HB + s0 + ns, 1:257],
                           W["b_ai"][64:128, 0:1], False)])

                ab_a = new_act("ab_a", 128, BSEG, 258, 1)
                conv(ab_in, "w_abc1", 114, 3, 256, BSEG, 2, 1,
                     [wfull(ab_a, 1, 256, bname="b_abc1")])
                ab_o = new_act("ab_o", 128, BSEG, 258, 0)
                conv(ab_a, "w_abc2", 114, 3, 256, BSEG, 2, 1,
                     [wfull(ab_o, 1, 256, bname="b_abc2")],
                     skip=(ab_in, 1))

                # ---- sub_pixel: ab_o -> ao_in (per-slot DMAs)
                ao_in = new_act("ao_in", 128, HB, 514, 1)
                for strm in (0, 1):
                    for j in (0, 1):
                        for k in (0, 1):
                            for s in range(HB):
                                nc.sync.dma_start(
                                    out=ao_in[64 * k + 32 * strm:
                                              64 * k + 32 * strm + 32,
                                              s, 1 + j:1 + j + 512:2],
                                    in_=ab_o[64 * strm + j:
                                             64 * strm + 64:2,
                                             HB * k + s, 1:257])

                ao0a = new_act("ao0a", 128, HB, 514, 1)
                conv(ao_in, "w_ao0c1", 121, 3, 512, HB, 1, 1,
                     [wfull(ao0a, 1, 512, bname="b_ao0c1")])
                ao0b = new_act("ao0b", 128, HB, 514, 1)
                conv(ao0a, "w_ao0c2", 121, 3, 512, HB, 1, 1,
                     [wfull(ao0b, 1, 512, bname="b_ao0c2")],
                     skip=(ao_in, 1))
                ao1a = new_act("ao1a", 128, HB, 514, 1)
                conv(ao0b, "w_ao1c1", 121, 3, 512, HB, 1, 1,
                     [wfull(ao1a, 1, 512, bname="b_ao1c1")])
                # ao1c2: 4-way scatter into dec_in
                dec_in = new_act("dec_in", 128, HB, 514, 1)
                conv(ao1a, "w_ao1c2", 121, 3, 512, HB, 1, 1,
                     [(0, 32, lambda s0, ns:
                       dec_in[0:32, s0:s0 + ns, 1:513],
                       W["b_ao1c2"][0:32, 0:1], True),
                      (32, 64, lambda s0, ns:
                       dec_in[64:96, s0:s0 + ns, 1:513],
                       W["b_ao1c2"][32:64, 0:1], True),
                      (64, 96, lambda s0, ns:
                       dec_in[32:64, s0:s0 + ns, 1:513],
                       W["b_ao1c2"][64:96, 0:1], True),
                      (96, 128, lambda s0, ns:
                       dec_in[96:128, s0:s0 + ns, 1:513],
                       W["b_ao1c2"][96:128, 0:1], True)],
                     skip=(ao0b, 1))

                d0a = new_act("d0a", 128, HB, 514, 1)
                conv(dec_in, "w_d0c1", 121, 3, 512, HB, 1, 1,
                     [wfull(d0a, 1, 512, bname="b_d0c1")])
                d0b = new_act("d0b", 128, HB, 514, 1)
                conv(d0a, "w_d0c2", 121, 3, 512, HB, 1, 1,
                     [wfull(d0b, 1, 512, bname="b_d0c2")], skip=(dec_in, 1))
                d1a = new_act("d1a", 128, HB, 514, 1)
                conv(d0b, "w_d1c1", 121, 3, 512, HB, 1, 1,
                     [wfull(d1a, 1, 512, bname="b_d1c1")])
                d1b = new_act("d1b", 128, HB, 514, 0)
                conv(d1a, "w_d1c2", 121, 3, 512, HB, 1, 1,
                     [wfull(d1b, 1, 512, bname="b_d1c2")], skip=(d0b, 1))

                # ---- transposes into actT
                for s in range(HB):
                    for lb in range(4):
                        ps_t = misc_ps.tile([128, 128], BF16, tag="mps")
                        nc.tensor.transpose(
                            ps_t[:, :], d1b[0:128, s, 1 + 128 * lb:
                                            1 + 128 * (lb + 1)],
                            W["ident"][:, :])
                        src4 = ps_t.rearrange("p (b c) -> p c b", c=32)
                        for sti, st in ((0, "s"), (1, "n")):
                            base_r = lb * 128 + sb + s
                            dst = actT[st].rearrange(
                                "p (c r) -> p c r", r=512)[
                                :, 0:25, base_r:base_r + HB + 1:HB]
                            nc.scalar.copy(
                                dst, src4[:, 0:25, 2 * sti:2 * sti + 2])

            # ---------------- fc
            ps_fc = {st: conv_ps.tile([128, 512], F32, tag="cps",
                                          name=f"psfc_{st}")
                     for st in ("s", "n")}
            for kc in range(100):
                wt = wfc_p.tile([128, 512], BF16, tag="fcw")
                nc.sync.dma_start(out=wt[:, :], in_=dram["fc_Wt"][kc, :, :])
                for st in ("s", "n"):
                    nc.tensor.matmul(ps_fc[st][:, :],
                                     actT[st][:, kc * 128:(kc + 1) * 128],
                                     wt[:, :], start=(kc == 0), stop=False,
                                     skip_group_check=True)
            fcb_t = wpool.tile([1, 512], BF16, tag="fcb")
            nc.sync.dma_start(out=fcb_t[:, :], in_=dram["fc_b"][:, :])
            for st, od in (("s", dec_s_d), ("n", dec_n_d)):
                nc.tensor.matmul(ps_fc[st][:, :], W["ones_1"][0:1, :],
                                 fcb_t[:, :], start=False, stop=True,
                                 skip_group_check=True)
                o_t = osb_p.tile([128, 512], F32, tag="osb")
                nc.scalar.activation(o_t[:, :], ps_fc[st][:, :], AF.Tanh)
                nc.sync.dma_start(out=od[0:kb.bcore, :],
                                  in_=o_t[0:kb.bcore, :])

    nc.finalize()
    return nc, kb


# ---------------------------------------------------------------- runner

_CACHE = {}


def _get_nc():
    if "nc" not in _CACHE:
        _CACHE["nc"] = build_nc(4, 32)
    return _CACHE["nc"]


def kernel(x, params):
    x = np.asarray(x, np.float32)
    nc, kb = _get_nc()
    H = prep_weights(params)
    in_maps = []
    for c in range(N_CORES):
        m = dict(H)
        m["x"] = x[c * B_CORE:(c + 1) * B_CORE].astype(ml_dtypes.bfloat16)
        in_maps.append(m)
    trace = os.environ.get("KERNEL_TRACE") == "1"
    res = run_bass_kernel_spmd(nc, in_maps, core_ids=list(range(N_CORES)),
                               trace=trace)
    _CACHE["last_res"] = res
    dec_s = np.concatenate([res.results[c]["dec_s"] for c in range(N_CORES)])
    dec_n = np.concatenate([res.results[c]["dec_n"] for c in range(N_CORES)])
    prob_s = np.concatenate(
        [np.transpose(res.results[c]["prob_s"], (1, 2, 0))
         for c in range(N_CORES)])
    prob_n = np.concatenate(
        [np.transpose(res.results[c]["prob_n"], (1, 2, 0))
         for c in range(N_CORES)])
    return dec_s, dec_n, prob_s, prob_n
